# revision 12
# baseline (speedup 1.0000x reference)
"""Dual-score attention kernel for Trainium2 (8 NeuronCores).

Computes, for inputs q_val/q_pos [B,L,H,E], k_val/k_pos/v_val/v_pos [B,S,H,E]:
    scores = einsum('blhe,bshe->bhls', q_val, k_val)
           + einsum('blhe,bshe->bhls', q_pos, k_pos)
    A  = softmax(scores / sqrt(E), axis=-1)
    V  = einsum('bhls,bshe->blhe', A, v_val)
    Vp = einsum('bhls,bshe->blhe', A, v_pos)
    returns (V, Vp, None)          # attn_mask is (faithfully) ignored

Sharding: the 16 (b,h) pairs are independent; each of the 8 cores handles 2.

Device-side layout trick: val/pos are concatenated along E (64+64=128) so the
dual-score sum is a single K=128 matmul. Scores are computed transposed
(St[s,l]) so that the softmax reduction over s lands on the PE partition dim,
where it is computed by a ones-vector matmul, and so that P=exp(St) is already
in the right layout to be the moving operand of the A@V matmul (stationary
Vcat = [v_val | v_pos], output [e'=128, l] accumulated over s tiles in PSUM).
"""

import os
import sys

import numpy as np

for _p in ("/opt/trn_rl_repo", "/root/.axon_site/_ro/trn_rl_repo"):
    if os.path.isdir(_p) and _p not in sys.path:
        sys.path.append(_p)

import ml_dtypes

import concourse.bass as bass
import concourse.tile as tile
from concourse import bacc, mybir
from concourse.bass_utils import run_bass_kernel_spmd

B, L, S, H, E = 2, 2048, 2048, 8, 64
NCORES = 8
NPAIR = 2          # (b,h) pairs per core
NT = S // 128      # 16 s-tiles
LH = 2             # l halves (PSUM budget)
LHW = L // LH      # 1024
NB = 512           # matmul free-dim chunk (one PSUM bank fp32)
SCALE = 1.0 / float(np.sqrt(E))

BF16 = mybir.dt.bfloat16
F32 = mybir.dt.float32

_CACHE = {}


def _build():
    nc = bacc.Bacc("TRN2", target_bir_lowering=False)

    qT = nc.dram_tensor("qT", [NPAIR, 128, L], BF16, kind="ExternalInput")
    kT = nc.dram_tensor("kT", [NPAIR, 128, S], BF16, kind="ExternalInput")
    vc = nc.dram_tensor("vc", [NPAIR, 128, NT, 128], BF16, kind="ExternalInput")
    out = nc.dram_tensor("out", [NPAIR, 128, L], F32, kind="ExternalOutput")

    with tile.TileContext(nc) as tc:
        with (
            tc.tile_pool(name="consts", bufs=1) as consts,
            tc.tile_pool(name="qk", bufs=2) as qk,
            tc.tile_pool(name="vpool", bufs=2) as vpool,
            tc.tile_pool(name="ppool", bufs=18) as ppool,
            tc.tile_pool(name="tailpool", bufs=2) as tailpool,
            tc.tile_pool(name="opool", bufs=2) as opool,
            tc.tile_pool(name="st_ps", bufs=2, space="PSUM") as st_ps,
            tc.tile_pool(name="pv_ps", bufs=2, space="PSUM") as pv_ps,
            tc.tile_pool(name="rs_ps", bufs=2, space="PSUM") as rs_ps,
        ):
            ones = consts.tile([128, 1], BF16, tag="ones")
            nc.vector.memset(ones[:], 1.0)
            # HAM warmup: junk matmuls keep the PE busy during the initial
            # DMA wait so real matmuls start at 2.4 GHz instead of 1.2.
            wz = consts.tile([128, 128], BF16, tag="wz")
            nc.vector.memset(wz[:], 0.0)
            for w in range(28):
                warm = rs_ps.tile([1, 128], F32, tag="rs", name=f"warm{w}")
                nc.tensor.matmul(warm[:], ones[:], wz[:], start=True, stop=True)

            # input tiles for both pairs, loaded upfront (double-buffered pools)
            qts, kts, vts = {}, {}, {}
            for pr in range(NPAIR):
                qt = qk.tile([128, L], BF16, tag="qt", name=f"qt{pr}")
                kt = qk.tile([128, S], BF16, tag="kt", name=f"kt{pr}")
                vt = vpool.tile([128, NT, 128], BF16, tag="vt", name=f"vt{pr}")
                qts[pr], kts[pr], vts[pr] = qt, kt, vt
                for c in range(4):
                    nc.sync.dma_start(
                        out=kt[:, c * 512:(c + 1) * 512],
                        in_=kT[pr, :, c * 512:(c + 1) * 512],
                    )
                for c in range(2):
                    nc.scalar.dma_start(
                        out=qt[:, c * LHW:(c + 1) * LHW],
                        in_=qT[pr, :, c * LHW:(c + 1) * LHW],
                    )
                for c in range(2):
                    nc.scalar.dma_start(
                        out=vt[:, 8 * c:8 * (c + 1), :],
                        in_=vc[pr, :, 8 * c:8 * (c + 1), :],
                    )

            # Global flat software pipeline over 64 (unit, i) slots.
            # Per slot: rowsum(g-4), pv(g-3), scores(g) — scores LAST so the
            # one genuinely coupled wait (st slot freed by exp(i-2)) lands
            # after the slot's independent work, and the ACT chain paces the
            # kernel (~1.28us/slot) with the PE saturated underneath.
            units = [(pr, lh) for pr in range(NPAIR) for lh in range(LH)]
            G = len(units) * NT
            PV_LAG, RS_LAG = 3, 4

            pvt, rst, pts = {}, {}, {}
            rrept, pvct = {}, {}

            def emit_sc(g):
                u, i = divmod(g, NT)
                pr, lh = units[u]
                p_i = ppool.tile([128, LHW], BF16, tag="p", name=f"p{u}_{i}")
                pts[(u, i)] = p_i
                st = st_ps.tile([128, LHW], F32, tag="st", name=f"st{u}_{i}")
                for c in range(2):
                    nc.tensor.matmul(
                        st[:, c * NB:(c + 1) * NB],
                        kts[pr][:, i * 128:(i + 1) * 128],
                        qts[pr][:, lh * LHW + c * NB: lh * LHW + (c + 1) * NB],
                        start=True,
                        stop=True,
                    )
                nc.scalar.activation(
                    p_i[:], st[:], mybir.ActivationFunctionType.Exp, scale=SCALE,
                )

            def emit_pv(g):
                u, i = divmod(g, NT)
                pr, _ = units[u]
                if i == 0:
                    pvt[u] = [
                        pv_ps.tile([128, NB], F32, tag="pv", name=f"pv{u}{c}")
                        for c in range(2)
                    ]
                for c in range(2):
                    nc.tensor.matmul(
                        pvt[u][c][:],
                        vts[pr][:, i, :],
                        pts[(u, i)][:, c * NB:(c + 1) * NB],
                        start=(i == 0),
                        stop=(i == NT - 1),
                    )
                if i == NT - 1:
                    # evacuate PSUM promptly so the next unit's pv banks free
                    pvc = tailpool.tile([128, LHW], F32, tag="pvc",
                                        name=f"pvc{u}")
                    pvct[u] = pvc
                    for c in range(2):
                        nc.vector.tensor_copy(
                            pvc[:, c * NB:(c + 1) * NB], pvt[u][c][:]
                        )

            def emit_rs(g):
                u, i = divmod(g, NT)
                if i == 0:
                    rst[u] = [
                        rs_ps.tile([1, NB], F32, tag="rs", name=f"rs{u}{c}")
                        for c in range(2)
                    ]
                for c in range(2):
                    nc.tensor.matmul(
                        rst[u][c][:],
                        ones[:],
                        pts[(u, i)][:, c * NB:(c + 1) * NB],
                        start=(i == 0),
                        stop=(i == NT - 1),
                    )
                if i == NT - 1:
                    recip = tailpool.tile([1, LHW], F32, tag="recip",
                                          name=f"recip{u}")
                    for c in range(2):
                        nc.vector.reciprocal_approx_fast(
                            out=recip[:, c * NB:(c + 1) * NB], in_=rst[u][c][:]
                        )
                    rrep = tailpool.tile([128, LHW], F32, tag="rrep",
                                         name=f"rrep{u}")
                    nc.gpsimd.partition_broadcast(rrep[:], recip[:])
                    rrept[u] = rrep

            def emit_tail(u):
                pr, lh = units[u]
                o_sb = opool.tile([128, LHW], F32, tag="o", name=f"o{u}")
                nc.vector.tensor_mul(o_sb[:], pvct[u][:], rrept[u][:])
                for c in range(2):
                    nc.sync.dma_start(
                        out=out[pr, :, lh * LHW + c * NB: lh * LHW + (c + 1) * NB],
                        in_=o_sb[:, c * NB:(c + 1) * NB],
                    )

            for g in range(G + RS_LAG + 1):
                h = g - RS_LAG
                if 0 <= h < G:
                    emit_rs(h)
                    u, i = divmod(h, NT)
                    if i == NT - 1:
                        emit_tail(u)
                h = g - PV_LAG
                if 0 <= h < G:
                    emit_pv(h)
                if g < G:
                    emit_sc(g)

    nc.compile()
    return nc


def _get_nc():
    if "nc" not in _CACHE:
        _CACHE["nc"] = _build()
    return _CACHE["nc"]


def _prep_inputs(q_val, q_pos, k_val, k_pos, v_val, v_pos):
    bf16 = ml_dtypes.bfloat16
    # [B,L,H,2E] -> [B,H,2E,L] -> [16, 128, L]
    qcat = np.concatenate([q_val, q_pos], axis=-1).transpose(0, 2, 3, 1)
    qcatT = np.ascontiguousarray(qcat.reshape(B * H, 2 * E, L)).astype(bf16)
    kcat = np.concatenate([k_val, k_pos], axis=-1).transpose(0, 2, 3, 1)
    kcatT = np.ascontiguousarray(kcat.reshape(B * H, 2 * E, S)).astype(bf16)
    # [B,S,H,2E] -> [B,H,S,2E] -> [16, NT, 128, 128] -> [16, 128(s), NT, 128(e)]
    vcat = np.concatenate([v_val, v_pos], axis=-1).transpose(0, 2, 1, 3)
    vtiles = vcat.reshape(B * H, NT, 128, 2 * E).transpose(0, 2, 1, 3)
    vtiles = np.ascontiguousarray(vtiles).astype(bf16)
    return [
        {
            "qT": np.ascontiguousarray(qcatT[2 * c: 2 * c + 2]),
            "kT": np.ascontiguousarray(kcatT[2 * c: 2 * c + 2]),
            "vc": np.ascontiguousarray(vtiles[2 * c: 2 * c + 2]),
        }
        for c in range(NCORES)
    ]


def kernel(q_val, q_pos, k_val, k_pos, v_val, v_pos, attn_mask=None):
    q_val, q_pos, k_val, k_pos, v_val, v_pos = (
        np.asarray(x, dtype=np.float32)
        for x in (q_val, q_pos, k_val, k_pos, v_val, v_pos)
    )
    nc = _get_nc()
    in_maps = _prep_inputs(q_val, q_pos, k_val, k_pos, v_val, v_pos)
    res = run_bass_kernel_spmd(nc, in_maps, core_ids=list(range(NCORES)))
    _CACHE["last_results"] = res
    outs = np.stack([np.asarray(r["out"], dtype=np.float32) for r in res.results])
    # [8, NPAIR, 128, L] -> [16, 128, L] -> [B, H, L, 128]
    o = outs.reshape(B * H, 2 * E, L).transpose(0, 2, 1).reshape(B, H, L, 2 * E)
    V = np.ascontiguousarray(o[..., :E].transpose(0, 2, 1, 3), dtype=np.float32)
    Vp = np.ascontiguousarray(o[..., E:].transpose(0, 2, 1, 3), dtype=np.float32)
    return (V, Vp, None)


# revision 16
# speedup vs baseline: 1.0070x; 1.0070x over previous
"""Dual-score attention kernel for Trainium2 (8 NeuronCores).

Computes, for inputs q_val/q_pos [B,L,H,E], k_val/k_pos/v_val/v_pos [B,S,H,E]:
    scores = einsum('blhe,bshe->bhls', q_val, k_val)
           + einsum('blhe,bshe->bhls', q_pos, k_pos)
    A  = softmax(scores / sqrt(E), axis=-1)
    V  = einsum('bhls,bshe->blhe', A, v_val)
    Vp = einsum('bhls,bshe->blhe', A, v_pos)
    returns (V, Vp, None)          # attn_mask is (faithfully) ignored

Sharding: the 16 (b,h) pairs are independent; each of the 8 cores handles 2.

Device-side layout trick: val/pos are concatenated along E (64+64=128) so the
dual-score sum is a single K=128 matmul. Scores are computed transposed
(St[s,l]) so that the softmax reduction over s lands on the PE partition dim,
where it is computed by a ones-vector matmul, and so that P=exp(St) is already
in the right layout to be the moving operand of the A@V matmul (stationary
Vcat = [v_val | v_pos], output [e'=128, l] accumulated over s tiles in PSUM).
"""

import os
import sys

import numpy as np

for _p in ("/opt/trn_rl_repo", "/root/.axon_site/_ro/trn_rl_repo"):
    if os.path.isdir(_p) and _p not in sys.path:
        sys.path.append(_p)

import ml_dtypes

import concourse.bass as bass
import concourse.tile as tile
from concourse import bacc, mybir
from concourse.bass_utils import run_bass_kernel_spmd

B, L, S, H, E = 2, 2048, 2048, 8, 64
NCORES = 8
NPAIR = 2          # (b,h) pairs per core
NT = S // 128      # 16 s-tiles
LH = 2             # l halves (PSUM budget)
LHW = L // LH      # 1024
NB = 512           # matmul free-dim chunk (one PSUM bank fp32)
SCALE = 1.0 / float(np.sqrt(E))

BF16 = mybir.dt.bfloat16
F32 = mybir.dt.float32

_CACHE = {}


def _build():
    nc = bacc.Bacc("TRN2", target_bir_lowering=False)

    qT = nc.dram_tensor("qT", [NPAIR, 128, L], BF16, kind="ExternalInput")
    kT = nc.dram_tensor("kT", [NPAIR, 128, S], BF16, kind="ExternalInput")
    vc = nc.dram_tensor("vc", [NPAIR, 128, NT, 128], BF16, kind="ExternalInput")
    out = nc.dram_tensor("out", [NPAIR, 128, L], F32, kind="ExternalOutput")

    with tile.TileContext(nc) as tc:
        with (
            tc.tile_pool(name="consts", bufs=1) as consts,
            tc.tile_pool(name="qk", bufs=2) as qk,
            tc.tile_pool(name="vpool", bufs=2) as vpool,
            tc.tile_pool(name="ppool", bufs=18) as ppool,
            tc.tile_pool(name="tailpool", bufs=2) as tailpool,
            tc.tile_pool(name="opool", bufs=2) as opool,
            tc.tile_pool(name="st_ps", bufs=2, space="PSUM") as st_ps,
            tc.tile_pool(name="pv_ps", bufs=2, space="PSUM") as pv_ps,
            tc.tile_pool(name="rs_ps", bufs=2, space="PSUM") as rs_ps,
        ):
            ones = consts.tile([128, 1], BF16, tag="ones")
            nc.vector.memset(ones[:], 1.0)
            # HAM warmup: junk matmuls keep the PE busy during the initial
            # DMA wait so real matmuls start at 2.4 GHz instead of 1.2.
            wz = consts.tile([128, 128], BF16, tag="wz")
            nc.vector.memset(wz[:], 0.0)
            for w in range(28):
                warm = rs_ps.tile([1, 128], F32, tag="rs", name=f"warm{w}")
                nc.tensor.matmul(warm[:], ones[:], wz[:], start=True, stop=True)

            # input tiles for both pairs, loaded upfront (double-buffered pools)
            qts, kts, vts = {}, {}, {}
            for pr in range(NPAIR):
                qt = qk.tile([128, L], BF16, tag="qt", name=f"qt{pr}")
                kt = qk.tile([128, S], BF16, tag="kt", name=f"kt{pr}")
                vt = vpool.tile([128, NT, 128], BF16, tag="vt", name=f"vt{pr}")
                qts[pr], kts[pr], vts[pr] = qt, kt, vt
                for c in range(4):
                    nc.sync.dma_start(
                        out=kt[:, c * 512:(c + 1) * 512],
                        in_=kT[pr, :, c * 512:(c + 1) * 512],
                    )
                for c in range(2):
                    nc.sync.dma_start(
                        out=qt[:, c * LHW:(c + 1) * LHW],
                        in_=qT[pr, :, c * LHW:(c + 1) * LHW],
                    )
                for c in range(2):
                    nc.gpsimd.dma_start(
                        out=vt[:, 8 * c:8 * (c + 1), :],
                        in_=vc[pr, :, 8 * c:8 * (c + 1), :],
                    )

            # Global flat software pipeline over 64 (unit, i) slots.
            # Per slot: rowsum(g-4), pv(g-3), scores(g) — scores LAST so the
            # one genuinely coupled wait (st slot freed by exp(i-2)) lands
            # after the slot's independent work, and the ACT chain paces the
            # kernel (~1.28us/slot) with the PE saturated underneath.
            units = [(pr, lh) for pr in range(NPAIR) for lh in range(LH)]
            G = len(units) * NT
            PV_LAG, RS_LAG = 4, 3

            pvt, rst, pts = {}, {}, {}
            rrept, pvct = {}, {}

            def emit_sc(g):
                u, i = divmod(g, NT)
                pr, lh = units[u]
                p_i = ppool.tile([128, LHW], BF16, tag="p", name=f"p{u}_{i}")
                pts[(u, i)] = p_i
                st = st_ps.tile([128, LHW], F32, tag="st", name=f"st{u}_{i}")
                for c in range(2):
                    nc.tensor.matmul(
                        st[:, c * NB:(c + 1) * NB],
                        kts[pr][:, i * 128:(i + 1) * 128],
                        qts[pr][:, lh * LHW + c * NB: lh * LHW + (c + 1) * NB],
                        start=True,
                        stop=True,
                    )
                nc.scalar.activation(
                    p_i[:], st[:], mybir.ActivationFunctionType.Exp, scale=SCALE,
                )

            def emit_pv(g):
                u, i = divmod(g, NT)
                pr, _ = units[u]
                if i == 0:
                    pvt[u] = [
                        pv_ps.tile([128, NB], F32, tag="pv", name=f"pv{u}{c}")
                        for c in range(2)
                    ]
                for c in range(2):
                    nc.tensor.matmul(
                        pvt[u][c][:],
                        vts[pr][:, i, :],
                        pts[(u, i)][:, c * NB:(c + 1) * NB],
                        start=(i == 0),
                        stop=(i == NT - 1),
                    )


            def emit_rs(g):
                u, i = divmod(g, NT)
                if i == 0:
                    rst[u] = [
                        rs_ps.tile([1, NB], F32, tag="rs", name=f"rs{u}{c}")
                        for c in range(2)
                    ]
                for c in range(2):
                    nc.tensor.matmul(
                        rst[u][c][:],
                        ones[:],
                        pts[(u, i)][:, c * NB:(c + 1) * NB],
                        start=(i == 0),
                        stop=(i == NT - 1),
                    )
                if i == NT - 1:
                    recip = tailpool.tile([1, LHW], F32, tag="recip",
                                          name=f"recip{u}")
                    rrep = tailpool.tile([128, LHW], F32, tag="rrep",
                                         name=f"rrep{u}")
                    for c in range(2):
                        nc.vector.reciprocal_approx_fast(
                            out=recip[:, c * NB:(c + 1) * NB], in_=rst[u][c][:]
                        )
                        nc.gpsimd.partition_broadcast(
                            rrep[:, c * NB:(c + 1) * NB],
                            recip[:, c * NB:(c + 1) * NB],
                        )
                    rrept[u] = rrep

            def emit_tail(u):
                # per-chunk copy->mult->dma chain; the copy also frees the
                # pv PSUM banks for the next unit
                pr, lh = units[u]
                pvc = tailpool.tile([128, LHW], F32, tag="pvc", name=f"pvc{u}")
                o_sb = opool.tile([128, LHW], F32, tag="o", name=f"o{u}")
                for c in range(2):
                    sl = slice(c * NB, (c + 1) * NB)
                    nc.vector.tensor_copy(pvc[:, sl], pvt[u][c][:])
                    nc.vector.tensor_mul(o_sb[:, sl], pvc[:, sl],
                                         rrept[u][:, sl])
                    nc.sync.dma_start(
                        out=out[pr, :, lh * LHW + c * NB: lh * LHW + (c + 1) * NB],
                        in_=o_sb[:, sl],
                    )

            for g in range(G + PV_LAG + 1):
                h = g - RS_LAG
                if 0 <= h < G:
                    emit_rs(h)
                h = g - PV_LAG
                if 0 <= h < G:
                    emit_pv(h)
                    u, i = divmod(h, NT)
                    if i == NT - 1:
                        emit_tail(u)
                if g < G:
                    emit_sc(g)

    nc.compile()
    return nc


def _get_nc():
    if "nc" not in _CACHE:
        _CACHE["nc"] = _build()
    return _CACHE["nc"]


def _prep_inputs(q_val, q_pos, k_val, k_pos, v_val, v_pos):
    bf16 = ml_dtypes.bfloat16
    # [B,L,H,2E] -> [B,H,2E,L] -> [16, 128, L]
    qcat = np.concatenate([q_val, q_pos], axis=-1).transpose(0, 2, 3, 1)
    qcatT = np.ascontiguousarray(qcat.reshape(B * H, 2 * E, L)).astype(bf16)
    kcat = np.concatenate([k_val, k_pos], axis=-1).transpose(0, 2, 3, 1)
    kcatT = np.ascontiguousarray(kcat.reshape(B * H, 2 * E, S)).astype(bf16)
    # [B,S,H,2E] -> [B,H,S,2E] -> [16, NT, 128, 128] -> [16, 128(s), NT, 128(e)]
    vcat = np.concatenate([v_val, v_pos], axis=-1).transpose(0, 2, 1, 3)
    vtiles = vcat.reshape(B * H, NT, 128, 2 * E).transpose(0, 2, 1, 3)
    vtiles = np.ascontiguousarray(vtiles).astype(bf16)
    return [
        {
            "qT": np.ascontiguousarray(qcatT[2 * c: 2 * c + 2]),
            "kT": np.ascontiguousarray(kcatT[2 * c: 2 * c + 2]),
            "vc": np.ascontiguousarray(vtiles[2 * c: 2 * c + 2]),
        }
        for c in range(NCORES)
    ]


def kernel(q_val, q_pos, k_val, k_pos, v_val, v_pos, attn_mask=None):
    q_val, q_pos, k_val, k_pos, v_val, v_pos = (
        np.asarray(x, dtype=np.float32)
        for x in (q_val, q_pos, k_val, k_pos, v_val, v_pos)
    )
    nc = _get_nc()
    in_maps = _prep_inputs(q_val, q_pos, k_val, k_pos, v_val, v_pos)
    res = run_bass_kernel_spmd(nc, in_maps, core_ids=list(range(NCORES)))
    _CACHE["last_results"] = res
    outs = np.stack([np.asarray(r["out"], dtype=np.float32) for r in res.results])
    # [8, NPAIR, 128, L] -> [16, 128, L] -> [B, H, L, 128]
    o = outs.reshape(B * H, 2 * E, L).transpose(0, 2, 1).reshape(B, H, L, 2 * E)
    V = np.ascontiguousarray(o[..., :E].transpose(0, 2, 1, 3), dtype=np.float32)
    Vp = np.ascontiguousarray(o[..., E:].transpose(0, 2, 1, 3), dtype=np.float32)
    return (V, Vp, None)


# revision 18
# speedup vs baseline: 1.0594x; 1.0520x over previous
"""Dual-score attention kernel for Trainium2 (8 NeuronCores).

Computes, for inputs q_val/q_pos [B,L,H,E], k_val/k_pos/v_val/v_pos [B,S,H,E]:
    scores = einsum('blhe,bshe->bhls', q_val, k_val)
           + einsum('blhe,bshe->bhls', q_pos, k_pos)
    A  = softmax(scores / sqrt(E), axis=-1)
    V  = einsum('bhls,bshe->blhe', A, v_val)
    Vp = einsum('bhls,bshe->blhe', A, v_pos)
    returns (V, Vp, None)          # attn_mask is (faithfully) ignored

Sharding: the 16 (b,h) pairs are independent; each of the 8 cores handles 2.

Device-side layout trick: val/pos are concatenated along E (64+64=128) so the
dual-score sum is a single K=128 matmul. Scores are computed transposed
(St[s,l]) so that the softmax reduction over s lands on the PE partition dim,
where it is computed by a ones-vector matmul, and so that P=exp(St) is already
in the right layout to be the moving operand of the A@V matmul (stationary
Vcat = [v_val | v_pos], output [e'=128, l] accumulated over s tiles in PSUM).
"""

import os
import sys

import numpy as np

for _p in ("/opt/trn_rl_repo", "/root/.axon_site/_ro/trn_rl_repo"):
    if os.path.isdir(_p) and _p not in sys.path:
        sys.path.append(_p)

import ml_dtypes

import concourse.bass as bass
import concourse.tile as tile
from concourse import bacc, mybir
from concourse.bass_utils import run_bass_kernel_spmd

B, L, S, H, E = 2, 2048, 2048, 8, 64
NCORES = 8
NPAIR = 2          # (b,h) pairs per core
NT = S // 128      # 16 s-tiles
LH = 2             # l halves (PSUM budget)
LHW = L // LH      # 1024
NB = 512           # matmul free-dim chunk (one PSUM bank fp32)
SCALE = 1.0 / float(np.sqrt(E))

BF16 = mybir.dt.bfloat16
F32 = mybir.dt.float32

_CACHE = {}


def _build():
    nc = bacc.Bacc("TRN2", target_bir_lowering=False)

    qT = nc.dram_tensor("qT", [NPAIR, 128, L], BF16, kind="ExternalInput")
    kT = nc.dram_tensor("kT", [NPAIR, 128, S], BF16, kind="ExternalInput")
    vc = nc.dram_tensor("vc", [NPAIR, 128, NT, 128], BF16, kind="ExternalInput")
    out = nc.dram_tensor("out", [NPAIR, 128, L], F32, kind="ExternalOutput")

    with tile.TileContext(nc) as tc:
        with (
            tc.tile_pool(name="consts", bufs=1) as consts,
            tc.tile_pool(name="qk", bufs=2) as qk,
            tc.tile_pool(name="vpool", bufs=2) as vpool,
            tc.tile_pool(name="ppool", bufs=18) as ppool,
            tc.tile_pool(name="tailpool", bufs=2) as tailpool,
            tc.tile_pool(name="opool", bufs=2) as opool,
            tc.tile_pool(name="st_ps", bufs=2, space="PSUM") as st_ps,
            tc.tile_pool(name="pv_ps", bufs=2, space="PSUM") as pv_ps,
            tc.tile_pool(name="rs_ps", bufs=2, space="PSUM") as rs_ps,
        ):
            ones = consts.tile([128, 1], BF16, tag="ones")
            nc.vector.memset(ones[:], 1.0)
            # HAM warmup: junk matmuls keep the PE busy during the initial
            # DMA wait so real matmuls start at 2.4 GHz instead of 1.2.
            wz = consts.tile([128, 128], BF16, tag="wz")
            nc.vector.memset(wz[:], 0.0)
            for w in range(12):
                warm = rs_ps.tile([1, 128], F32, tag="rs", name=f"warm{w}")
                nc.tensor.matmul(warm[:], ones[:], wz[:], start=True, stop=True)

            # input tiles for both pairs, loaded upfront (double-buffered pools)
            qts, kts, vts = {}, {}, {}
            for pr in range(NPAIR):
                qt = qk.tile([128, L], BF16, tag="qt", name=f"qt{pr}")
                kt = qk.tile([128, S], BF16, tag="kt", name=f"kt{pr}")
                vt = vpool.tile([128, NT, 128], BF16, tag="vt", name=f"vt{pr}")
                qts[pr], kts[pr], vts[pr] = qt, kt, vt
                # one queue, ordered by first use: kt c0 + qt c0 unblock the
                # first scores matmul ~2us in; v tiles aren't needed until
                # the first pv pass
                nc.sync.dma_start(out=kt[:, 0:512], in_=kT[pr, :, 0:512])
                nc.sync.dma_start(out=qt[:, 0:LHW], in_=qT[pr, :, 0:LHW])
                for c in range(1, 4):
                    nc.sync.dma_start(
                        out=kt[:, c * 512:(c + 1) * 512],
                        in_=kT[pr, :, c * 512:(c + 1) * 512],
                    )
                nc.sync.dma_start(out=qt[:, LHW:L], in_=qT[pr, :, LHW:L])
                for c in range(2):
                    nc.sync.dma_start(
                        out=vt[:, 8 * c:8 * (c + 1), :],
                        in_=vc[pr, :, 8 * c:8 * (c + 1), :],
                    )

            # Global flat software pipeline over 64 (unit, i) slots.
            # Per slot: rowsum(g-4), pv(g-3), scores(g) — scores LAST so the
            # one genuinely coupled wait (st slot freed by exp(i-2)) lands
            # after the slot's independent work, and the ACT chain paces the
            # kernel (~1.28us/slot) with the PE saturated underneath.
            units = [(pr, lh) for pr in range(NPAIR) for lh in range(LH)]
            G = len(units) * NT
            PV_LAG, RS_LAG = 4, 3

            pvt, rst, pts = {}, {}, {}
            rrept, pvct = {}, {}

            def emit_sc(g):
                u, i = divmod(g, NT)
                pr, lh = units[u]
                p_i = ppool.tile([128, LHW], BF16, tag="p", name=f"p{u}_{i}")
                pts[(u, i)] = p_i
                st = st_ps.tile([128, LHW], F32, tag="st", name=f"st{u}_{i}")
                for c in range(2):
                    nc.tensor.matmul(
                        st[:, c * NB:(c + 1) * NB],
                        kts[pr][:, i * 128:(i + 1) * 128],
                        qts[pr][:, lh * LHW + c * NB: lh * LHW + (c + 1) * NB],
                        start=True,
                        stop=True,
                    )
                nc.scalar.activation(
                    p_i[:], st[:], mybir.ActivationFunctionType.Exp, scale=SCALE,
                )

            def emit_pv(g):
                u, i = divmod(g, NT)
                pr, _ = units[u]
                if i == 0:
                    pvt[u] = [
                        pv_ps.tile([128, NB], F32, tag="pv", name=f"pv{u}{c}")
                        for c in range(2)
                    ]
                for c in range(2):
                    nc.tensor.matmul(
                        pvt[u][c][:],
                        vts[pr][:, i, :],
                        pts[(u, i)][:, c * NB:(c + 1) * NB],
                        start=(i == 0),
                        stop=(i == NT - 1),
                    )


            def emit_rs(g):
                u, i = divmod(g, NT)
                if i == 0:
                    rst[u] = [
                        rs_ps.tile([1, NB], F32, tag="rs", name=f"rs{u}{c}")
                        for c in range(2)
                    ]
                for c in range(2):
                    nc.tensor.matmul(
                        rst[u][c][:],
                        ones[:],
                        pts[(u, i)][:, c * NB:(c + 1) * NB],
                        start=(i == 0),
                        stop=(i == NT - 1),
                    )
                if i == NT - 1:
                    recip = tailpool.tile([1, LHW], F32, tag="recip",
                                          name=f"recip{u}")
                    rrep = tailpool.tile([128, LHW], F32, tag="rrep",
                                         name=f"rrep{u}")
                    for c in range(2):
                        nc.vector.reciprocal_approx_fast(
                            out=recip[:, c * NB:(c + 1) * NB], in_=rst[u][c][:]
                        )
                        nc.gpsimd.partition_broadcast(
                            rrep[:, c * NB:(c + 1) * NB],
                            recip[:, c * NB:(c + 1) * NB],
                        )
                    rrept[u] = rrep

            def emit_tail(u):
                # per-chunk copy->mult->dma chain; the copy also frees the
                # pv PSUM banks for the next unit
                pr, lh = units[u]
                pvc = tailpool.tile([128, LHW], F32, tag="pvc", name=f"pvc{u}")
                o_sb = opool.tile([128, LHW], F32, tag="o", name=f"o{u}")
                for c in range(2):
                    sl = slice(c * NB, (c + 1) * NB)
                    nc.vector.tensor_copy(pvc[:, sl], pvt[u][c][:])
                    nc.vector.tensor_mul(o_sb[:, sl], pvc[:, sl],
                                         rrept[u][:, sl])
                    nc.sync.dma_start(
                        out=out[pr, :, lh * LHW + c * NB: lh * LHW + (c + 1) * NB],
                        in_=o_sb[:, sl],
                    )

            for g in range(G + PV_LAG + 1):
                h = g - RS_LAG
                if 0 <= h < G:
                    emit_rs(h)
                h = g - PV_LAG
                if 0 <= h < G:
                    emit_pv(h)
                    u, i = divmod(h, NT)
                    if i == NT - 1:
                        emit_tail(u)
                if g < G:
                    emit_sc(g)

    nc.compile()
    return nc


def _get_nc():
    if "nc" not in _CACHE:
        _CACHE["nc"] = _build()
    return _CACHE["nc"]


def _prep_inputs(q_val, q_pos, k_val, k_pos, v_val, v_pos):
    bf16 = ml_dtypes.bfloat16
    # [B,L,H,2E] -> [B,H,2E,L] -> [16, 128, L]
    qcat = np.concatenate([q_val, q_pos], axis=-1).transpose(0, 2, 3, 1)
    qcatT = np.ascontiguousarray(qcat.reshape(B * H, 2 * E, L)).astype(bf16)
    kcat = np.concatenate([k_val, k_pos], axis=-1).transpose(0, 2, 3, 1)
    kcatT = np.ascontiguousarray(kcat.reshape(B * H, 2 * E, S)).astype(bf16)
    # [B,S,H,2E] -> [B,H,S,2E] -> [16, NT, 128, 128] -> [16, 128(s), NT, 128(e)]
    vcat = np.concatenate([v_val, v_pos], axis=-1).transpose(0, 2, 1, 3)
    vtiles = vcat.reshape(B * H, NT, 128, 2 * E).transpose(0, 2, 1, 3)
    vtiles = np.ascontiguousarray(vtiles).astype(bf16)
    return [
        {
            "qT": np.ascontiguousarray(qcatT[2 * c: 2 * c + 2]),
            "kT": np.ascontiguousarray(kcatT[2 * c: 2 * c + 2]),
            "vc": np.ascontiguousarray(vtiles[2 * c: 2 * c + 2]),
        }
        for c in range(NCORES)
    ]


def kernel(q_val, q_pos, k_val, k_pos, v_val, v_pos, attn_mask=None):
    q_val, q_pos, k_val, k_pos, v_val, v_pos = (
        np.asarray(x, dtype=np.float32)
        for x in (q_val, q_pos, k_val, k_pos, v_val, v_pos)
    )
    nc = _get_nc()
    in_maps = _prep_inputs(q_val, q_pos, k_val, k_pos, v_val, v_pos)
    res = run_bass_kernel_spmd(nc, in_maps, core_ids=list(range(NCORES)))
    _CACHE["last_results"] = res
    outs = np.stack([np.asarray(r["out"], dtype=np.float32) for r in res.results])
    # [8, NPAIR, 128, L] -> [16, 128, L] -> [B, H, L, 128]
    o = outs.reshape(B * H, 2 * E, L).transpose(0, 2, 1).reshape(B, H, L, 2 * E)
    V = np.ascontiguousarray(o[..., :E].transpose(0, 2, 1, 3), dtype=np.float32)
    Vp = np.ascontiguousarray(o[..., E:].transpose(0, 2, 1, 3), dtype=np.float32)
    return (V, Vp, None)


# revision 19
# speedup vs baseline: 1.1154x; 1.0529x over previous
"""Dual-score attention kernel for Trainium2 (8 NeuronCores).

Computes, for inputs q_val/q_pos [B,L,H,E], k_val/k_pos/v_val/v_pos [B,S,H,E]:
    scores = einsum('blhe,bshe->bhls', q_val, k_val)
           + einsum('blhe,bshe->bhls', q_pos, k_pos)
    A  = softmax(scores / sqrt(E), axis=-1)
    V  = einsum('bhls,bshe->blhe', A, v_val)
    Vp = einsum('bhls,bshe->blhe', A, v_pos)
    returns (V, Vp, None)          # attn_mask is (faithfully) ignored

Sharding: the 16 (b,h) pairs are independent; each of the 8 cores handles 2.

Device-side layout trick: val/pos are concatenated along E (64+64=128) so the
dual-score sum is a single K=128 matmul. Scores are computed transposed
(St[s,l]) so that the softmax reduction over s lands on the PE partition dim,
where it is computed by a ones-vector matmul, and so that P=exp(St) is already
in the right layout to be the moving operand of the A@V matmul (stationary
Vcat = [v_val | v_pos], output [e'=128, l] accumulated over s tiles in PSUM).
"""

import os
import sys

import numpy as np

for _p in ("/opt/trn_rl_repo", "/root/.axon_site/_ro/trn_rl_repo"):
    if os.path.isdir(_p) and _p not in sys.path:
        sys.path.append(_p)

import ml_dtypes

import concourse.bass as bass
import concourse.tile as tile
from concourse import bacc, mybir
from concourse.bass_utils import run_bass_kernel_spmd

B, L, S, H, E = 2, 2048, 2048, 8, 64
NCORES = 8
NPAIR = 2          # (b,h) pairs per core
NT = S // 128      # 16 s-tiles
LH = 2             # l halves (PSUM budget)
LHW = L // LH      # 1024
NB = 512           # matmul free-dim chunk (one PSUM bank fp32)
SCALE = 1.0 / float(np.sqrt(E))

BF16 = mybir.dt.bfloat16
F32 = mybir.dt.float32

_CACHE = {}


def _build():
    nc = bacc.Bacc("TRN2", target_bir_lowering=False)

    qT = nc.dram_tensor("qT", [NPAIR, 128, L], BF16, kind="ExternalInput")
    kT = nc.dram_tensor("kT", [NPAIR, 128, S], BF16, kind="ExternalInput")
    vc = nc.dram_tensor("vc", [NPAIR, 128, NT, 128], BF16, kind="ExternalInput")
    out = nc.dram_tensor("out", [NPAIR, 128, L], F32, kind="ExternalOutput")

    with tile.TileContext(nc) as tc:
        with (
            tc.tile_pool(name="consts", bufs=1) as consts,
            tc.tile_pool(name="qk", bufs=2) as qk,
            tc.tile_pool(name="vpool", bufs=2) as vpool,
            tc.tile_pool(name="ppool", bufs=18) as ppool,
            tc.tile_pool(name="tailpool", bufs=2) as tailpool,
            tc.tile_pool(name="opool", bufs=2) as opool,
            tc.tile_pool(name="st_ps", bufs=2, space="PSUM") as st_ps,
            tc.tile_pool(name="pv_ps", bufs=2, space="PSUM") as pv_ps,
            tc.tile_pool(name="rs_ps", bufs=2, space="PSUM") as rs_ps,
        ):
            ones = consts.tile([128, 1], BF16, tag="ones")
            nc.vector.memset(ones[:], 1.0)
            # HAM warmup: junk matmuls keep the PE busy during the initial
            # DMA wait so real matmuls start at 2.4 GHz instead of 1.2.
            wz = consts.tile([128, 128], BF16, tag="wz")
            nc.vector.memset(wz[:], 0.0)
            for w in range(12):
                warm = rs_ps.tile([1, 128], F32, tag="rs", name=f"warm{w}")
                nc.tensor.matmul(warm[:], ones[:], wz[:], start=True, stop=True)

            # input tiles for both pairs, loaded upfront (double-buffered pools)
            qts, kts, vts = {}, {}, {}
            for pr in range(NPAIR):
                qt = qk.tile([128, L], BF16, tag="qt", name=f"qt{pr}")
                kt = qk.tile([128, S], BF16, tag="kt", name=f"kt{pr}")
                vt = vpool.tile([128, NT, 128], BF16, tag="vt", name=f"vt{pr}")
                qts[pr], kts[pr], vts[pr] = qt, kt, vt
                # one queue, ordered by first use: kt c0 + qt c0 unblock the
                # first scores matmul ~2us in; v tiles aren't needed until
                # the first pv pass
                nc.sync.dma_start(out=kt[:, 0:512], in_=kT[pr, :, 0:512])
                nc.sync.dma_start(out=qt[:, 0:LHW], in_=qT[pr, :, 0:LHW])
                for c in range(1, 4):
                    nc.sync.dma_start(
                        out=kt[:, c * 512:(c + 1) * 512],
                        in_=kT[pr, :, c * 512:(c + 1) * 512],
                    )
                nc.sync.dma_start(out=qt[:, LHW:L], in_=qT[pr, :, LHW:L])
                for c in range(2):
                    nc.sync.dma_start(
                        out=vt[:, 8 * c:8 * (c + 1), :],
                        in_=vc[pr, :, 8 * c:8 * (c + 1), :],
                    )

            # Global flat software pipeline over 64 (unit, i) slots.
            # Per slot: rowsum(g-4), pv(g-3), scores(g) — scores LAST so the
            # one genuinely coupled wait (st slot freed by exp(i-2)) lands
            # after the slot's independent work, and the ACT chain paces the
            # kernel (~1.28us/slot) with the PE saturated underneath.
            units = [(pr, lh) for pr in range(NPAIR) for lh in range(LH)]
            G = len(units) * NT
            PV_LAG, RS_LAG = 4, 3

            pvt, rst, pts = {}, {}, {}
            rrept, pvct = {}, {}

            def emit_sc(g):
                u, i = divmod(g, NT)
                pr, lh = units[u]
                p_i = ppool.tile([128, LHW], BF16, tag="p", name=f"p{u}_{i}")
                pts[(u, i)] = p_i
                st = st_ps.tile([128, LHW], F32, tag="st", name=f"st{u}_{i}")
                for c in range(2):
                    nc.tensor.matmul(
                        st[:, c * NB:(c + 1) * NB],
                        kts[pr][:, i * 128:(i + 1) * 128],
                        qts[pr][:, lh * LHW + c * NB: lh * LHW + (c + 1) * NB],
                        start=True,
                        stop=True,
                    )
                nc.scalar.activation(
                    p_i[:], st[:], mybir.ActivationFunctionType.Exp, scale=SCALE,
                )

            def emit_pv(g):
                u, i = divmod(g, NT)
                pr, _ = units[u]
                if i == 0:
                    pvt[u] = [
                        pv_ps.tile([128, NB], F32, tag="pv", name=f"pv{u}{c}")
                        for c in range(2)
                    ]
                for c in range(2):
                    nc.tensor.matmul(
                        pvt[u][c][:],
                        vts[pr][:, i, :],
                        pts[(u, i)][:, c * NB:(c + 1) * NB],
                        start=(i == 0),
                        stop=(i == NT - 1),
                    )


            def emit_rs(g):
                u, i = divmod(g, NT)
                if i == 0:
                    rst[u] = [
                        rs_ps.tile([1, NB], F32, tag="rs", name=f"rs{u}{c}")
                        for c in range(2)
                    ]
                for c in range(2):
                    nc.tensor.matmul(
                        rst[u][c][:],
                        ones[:],
                        pts[(u, i)][:, c * NB:(c + 1) * NB],
                        start=(i == 0),
                        stop=(i == NT - 1),
                    )
                if i == NT - 1:
                    recip = tailpool.tile([1, LHW], F32, tag="recip",
                                          name=f"recip{u}")
                    rrep = tailpool.tile([128, LHW], F32, tag="rrep",
                                         name=f"rrep{u}")
                    for c in range(2):
                        nc.vector.reciprocal_approx_fast(
                            out=recip[:, c * NB:(c + 1) * NB], in_=rst[u][c][:]
                        )
                        nc.gpsimd.partition_broadcast(
                            rrep[:, c * NB:(c + 1) * NB],
                            recip[:, c * NB:(c + 1) * NB],
                        )
                    rrept[u] = rrep

            def emit_tail(u):
                # per-chunk copy->mult->dma chain; the copy also frees the
                # pv PSUM banks for the next unit
                pr, lh = units[u]
                pvc = tailpool.tile([128, LHW], F32, tag="pvc", name=f"pvc{u}")
                o_sb = opool.tile([128, LHW], F32, tag="o", name=f"o{u}")
                for c in range(2):
                    sl = slice(c * NB, (c + 1) * NB)
                    nc.vector.tensor_copy(pvc[:, sl], pvt[u][c][:])
                    nc.vector.tensor_mul(o_sb[:, sl], pvc[:, sl],
                                         rrept[u][:, sl])
                    nc.sync.dma_start(
                        out=out[pr, :, lh * LHW + c * NB: lh * LHW + (c + 1) * NB],
                        in_=o_sb[:, sl],
                    )

            for g in range(G + PV_LAG + 1):
                h = g - RS_LAG
                # pair the rowsums of two adjacent slots: 4 consecutive
                # matmuls share the `ones` stationary -> one fewer weight
                # swap per 2 slots
                if 0 <= h < G and h % 2 == 0:
                    emit_rs(h)
                    emit_rs(h + 1)
                h = g - PV_LAG
                if 0 <= h < G:
                    emit_pv(h)
                    u, i = divmod(h, NT)
                    if i == NT - 1:
                        emit_tail(u)
                if g < G:
                    emit_sc(g)

    nc.compile()
    return nc


def _get_nc():
    if "nc" not in _CACHE:
        _CACHE["nc"] = _build()
    return _CACHE["nc"]


def _prep_inputs(q_val, q_pos, k_val, k_pos, v_val, v_pos):
    bf16 = ml_dtypes.bfloat16
    # [B,L,H,2E] -> [B,H,2E,L] -> [16, 128, L]
    qcat = np.concatenate([q_val, q_pos], axis=-1).transpose(0, 2, 3, 1)
    qcatT = np.ascontiguousarray(qcat.reshape(B * H, 2 * E, L)).astype(bf16)
    kcat = np.concatenate([k_val, k_pos], axis=-1).transpose(0, 2, 3, 1)
    kcatT = np.ascontiguousarray(kcat.reshape(B * H, 2 * E, S)).astype(bf16)
    # [B,S,H,2E] -> [B,H,S,2E] -> [16, NT, 128, 128] -> [16, 128(s), NT, 128(e)]
    vcat = np.concatenate([v_val, v_pos], axis=-1).transpose(0, 2, 1, 3)
    vtiles = vcat.reshape(B * H, NT, 128, 2 * E).transpose(0, 2, 1, 3)
    vtiles = np.ascontiguousarray(vtiles).astype(bf16)
    return [
        {
            "qT": np.ascontiguousarray(qcatT[2 * c: 2 * c + 2]),
            "kT": np.ascontiguousarray(kcatT[2 * c: 2 * c + 2]),
            "vc": np.ascontiguousarray(vtiles[2 * c: 2 * c + 2]),
        }
        for c in range(NCORES)
    ]


def kernel(q_val, q_pos, k_val, k_pos, v_val, v_pos, attn_mask=None):
    q_val, q_pos, k_val, k_pos, v_val, v_pos = (
        np.asarray(x, dtype=np.float32)
        for x in (q_val, q_pos, k_val, k_pos, v_val, v_pos)
    )
    nc = _get_nc()
    in_maps = _prep_inputs(q_val, q_pos, k_val, k_pos, v_val, v_pos)
    res = run_bass_kernel_spmd(nc, in_maps, core_ids=list(range(NCORES)))
    _CACHE["last_results"] = res
    outs = np.stack([np.asarray(r["out"], dtype=np.float32) for r in res.results])
    # [8, NPAIR, 128, L] -> [16, 128, L] -> [B, H, L, 128]
    o = outs.reshape(B * H, 2 * E, L).transpose(0, 2, 1).reshape(B, H, L, 2 * E)
    V = np.ascontiguousarray(o[..., :E].transpose(0, 2, 1, 3), dtype=np.float32)
    Vp = np.ascontiguousarray(o[..., E:].transpose(0, 2, 1, 3), dtype=np.float32)
    return (V, Vp, None)


# revision 21
# speedup vs baseline: 1.1186x; 1.0029x over previous
"""Dual-score attention kernel for Trainium2 (8 NeuronCores).

Computes, for inputs q_val/q_pos [B,L,H,E], k_val/k_pos/v_val/v_pos [B,S,H,E]:
    scores = einsum('blhe,bshe->bhls', q_val, k_val)
           + einsum('blhe,bshe->bhls', q_pos, k_pos)
    A  = softmax(scores / sqrt(E), axis=-1)
    V  = einsum('bhls,bshe->blhe', A, v_val)
    Vp = einsum('bhls,bshe->blhe', A, v_pos)
    returns (V, Vp, None)          # attn_mask is (faithfully) ignored

Sharding: the 16 (b,h) pairs are independent; each of the 8 cores handles 2.

Device-side layout trick: val/pos are concatenated along E (64+64=128) so the
dual-score sum is a single K=128 matmul. Scores are computed transposed
(St[s,l]) so that the softmax reduction over s lands on the PE partition dim,
where it is computed by a ones-vector matmul, and so that P=exp(St) is already
in the right layout to be the moving operand of the A@V matmul (stationary
Vcat = [v_val | v_pos], output [e'=128, l] accumulated over s tiles in PSUM).
"""

import os
import sys

import numpy as np

for _p in ("/opt/trn_rl_repo", "/root/.axon_site/_ro/trn_rl_repo"):
    if os.path.isdir(_p) and _p not in sys.path:
        sys.path.append(_p)

import ml_dtypes

import concourse.bass as bass
import concourse.tile as tile
from concourse import bacc, mybir
from concourse.bass_utils import run_bass_kernel_spmd

B, L, S, H, E = 2, 2048, 2048, 8, 64
NCORES = 8
NPAIR = 2          # (b,h) pairs per core
NT = S // 128      # 16 s-tiles
LH = 2             # l halves (PSUM budget)
LHW = L // LH      # 1024
NB = 512           # matmul free-dim chunk (one PSUM bank fp32)
SCALE = 1.0 / float(np.sqrt(E))

BF16 = mybir.dt.bfloat16
F32 = mybir.dt.float32

_CACHE = {}


def _build():
    nc = bacc.Bacc("TRN2", target_bir_lowering=False)

    qT = nc.dram_tensor("qT", [NPAIR, 128, L], BF16, kind="ExternalInput")
    kT = nc.dram_tensor("kT", [NPAIR, 128, S], BF16, kind="ExternalInput")
    vc = nc.dram_tensor("vc", [NPAIR, 128, NT, 128], BF16, kind="ExternalInput")
    out = nc.dram_tensor("out", [NPAIR, 128, L], F32, kind="ExternalOutput")

    with tile.TileContext(nc) as tc:
        with (
            tc.tile_pool(name="consts", bufs=1) as consts,
            tc.tile_pool(name="qk", bufs=2) as qk,
            tc.tile_pool(name="vpool", bufs=2) as vpool,
            tc.tile_pool(name="ppool", bufs=18) as ppool,
            tc.tile_pool(name="tailpool", bufs=2) as tailpool,
            tc.tile_pool(name="opool", bufs=2) as opool,
            tc.tile_pool(name="st_ps", bufs=2, space="PSUM") as st_ps,
            tc.tile_pool(name="pv_ps", bufs=2, space="PSUM") as pv_ps,
            tc.tile_pool(name="rs_ps", bufs=2, space="PSUM") as rs_ps,
        ):
            ones = consts.tile([128, 1], BF16, tag="ones")
            nc.vector.memset(ones[:], 1.0)
            # HAM warmup: junk matmuls keep the PE busy during the initial
            # DMA wait so real matmuls start at 2.4 GHz instead of 1.2.
            wz = consts.tile([128, 128], BF16, tag="wz")
            nc.vector.memset(wz[:], 0.0)
            for w in range(12):
                warm = rs_ps.tile([1, 128], F32, tag="rs", name=f"warm{w}")
                nc.tensor.matmul(warm[:], ones[:], wz[:], start=True, stop=True)

            # input tiles for both pairs, loaded upfront (double-buffered pools)
            qts, kts, vts = {}, {}, {}
            for pr in range(NPAIR):
                qt = qk.tile([128, L], BF16, tag="qt", name=f"qt{pr}")
                kt = qk.tile([128, S], BF16, tag="kt", name=f"kt{pr}")
                vt = vpool.tile([128, NT, 128], BF16, tag="vt", name=f"vt{pr}")
                qts[pr], kts[pr], vts[pr] = qt, kt, vt
                # one queue, ordered by first use: kt c0 + qt c0 unblock the
                # first scores matmul ~2us in; vt halves arrive before the
                # pipelined pv pass reaches them
                nc.sync.dma_start(out=kt[:, 0:512], in_=kT[pr, :, 0:512])
                nc.sync.dma_start(out=qt[:, 0:LHW], in_=qT[pr, :, 0:LHW])
                nc.sync.dma_start(out=vt[:, 0:8, :], in_=vc[pr, :, 0:8, :])
                nc.sync.dma_start(out=kt[:, 512:1024], in_=kT[pr, :, 512:1024])
                nc.sync.dma_start(out=vt[:, 8:16, :], in_=vc[pr, :, 8:16, :])
                for c in range(2, 4):
                    nc.sync.dma_start(
                        out=kt[:, c * 512:(c + 1) * 512],
                        in_=kT[pr, :, c * 512:(c + 1) * 512],
                    )
                nc.sync.dma_start(out=qt[:, LHW:L], in_=qT[pr, :, LHW:L])

            # Global flat software pipeline over 64 (unit, i) slots.
            # Per slot: rowsum(g-4), pv(g-3), scores(g) — scores LAST so the
            # one genuinely coupled wait (st slot freed by exp(i-2)) lands
            # after the slot's independent work, and the ACT chain paces the
            # kernel (~1.28us/slot) with the PE saturated underneath.
            units = [(pr, lh) for pr in range(NPAIR) for lh in range(LH)]
            G = len(units) * NT
            PV_LAG, RS_LAG = 4, 3

            pvt, rst, pts = {}, {}, {}
            rrept, pvct = {}, {}

            def emit_sc(g):
                u, i = divmod(g, NT)
                pr, lh = units[u]
                p_i = ppool.tile([128, LHW], BF16, tag="p", name=f"p{u}_{i}")
                pts[(u, i)] = p_i
                st = st_ps.tile([128, LHW], F32, tag="st", name=f"st{u}_{i}")
                for c in range(2):
                    nc.tensor.matmul(
                        st[:, c * NB:(c + 1) * NB],
                        kts[pr][:, i * 128:(i + 1) * 128],
                        qts[pr][:, lh * LHW + c * NB: lh * LHW + (c + 1) * NB],
                        start=True,
                        stop=True,
                    )
                nc.scalar.activation(
                    p_i[:], st[:], mybir.ActivationFunctionType.Exp, scale=SCALE,
                )

            def emit_pv(g):
                u, i = divmod(g, NT)
                pr, _ = units[u]
                if i == 0:
                    pvt[u] = [
                        pv_ps.tile([128, NB], F32, tag="pv", name=f"pv{u}{c}")
                        for c in range(2)
                    ]
                for c in range(2):
                    nc.tensor.matmul(
                        pvt[u][c][:],
                        vts[pr][:, i, :],
                        pts[(u, i)][:, c * NB:(c + 1) * NB],
                        start=(i == 0),
                        stop=(i == NT - 1),
                    )


            def emit_rs(g):
                u, i = divmod(g, NT)
                if i == 0:
                    rst[u] = [
                        rs_ps.tile([1, NB], F32, tag="rs", name=f"rs{u}{c}")
                        for c in range(2)
                    ]
                for c in range(2):
                    nc.tensor.matmul(
                        rst[u][c][:],
                        ones[:],
                        pts[(u, i)][:, c * NB:(c + 1) * NB],
                        start=(i == 0),
                        stop=(i == NT - 1),
                    )
                if i == NT - 1:
                    recip = tailpool.tile([1, LHW], F32, tag="recip",
                                          name=f"recip{u}")
                    rrep = tailpool.tile([128, LHW], F32, tag="rrep",
                                         name=f"rrep{u}")
                    for c in range(2):
                        nc.vector.reciprocal_approx_fast(
                            out=recip[:, c * NB:(c + 1) * NB], in_=rst[u][c][:]
                        )
                        nc.gpsimd.partition_broadcast(
                            rrep[:, c * NB:(c + 1) * NB],
                            recip[:, c * NB:(c + 1) * NB],
                        )
                    rrept[u] = rrep

            def emit_tail(u):
                # per-chunk copy->mult->dma chain; the copy frees the pv
                # PSUM banks for the next unit. The last unit has no
                # successor, so it multiplies straight from PSUM.
                pr, lh = units[u]
                last = u == len(units) - 1
                o_sb = opool.tile([128, LHW], F32, tag="o", name=f"o{u}")
                if not last:
                    pvc = tailpool.tile([128, LHW], F32, tag="pvc",
                                        name=f"pvc{u}")
                for c in range(2):
                    sl = slice(c * NB, (c + 1) * NB)
                    if last:
                        nc.vector.tensor_mul(o_sb[:, sl], pvt[u][c][:],
                                             rrept[u][:, sl])
                    else:
                        nc.vector.tensor_copy(pvc[:, sl], pvt[u][c][:])
                        nc.vector.tensor_mul(o_sb[:, sl], pvc[:, sl],
                                             rrept[u][:, sl])
                    nc.sync.dma_start(
                        out=out[pr, :, lh * LHW + c * NB: lh * LHW + (c + 1) * NB],
                        in_=o_sb[:, sl],
                    )

            for g in range(G + PV_LAG + 1):
                h = g - RS_LAG
                # pair the rowsums of two adjacent slots: 4 consecutive
                # matmuls share the `ones` stationary -> one fewer weight
                # swap per 2 slots
                if 0 <= h < G and h % 2 == 0:
                    emit_rs(h)
                    emit_rs(h + 1)
                h = g - PV_LAG
                if 0 <= h < G:
                    emit_pv(h)
                    u, i = divmod(h, NT)
                    if i == NT - 1:
                        emit_tail(u)
                if g < G:
                    emit_sc(g)

    nc.compile()
    return nc


def _get_nc():
    if "nc" not in _CACHE:
        _CACHE["nc"] = _build()
    return _CACHE["nc"]


def _prep_inputs(q_val, q_pos, k_val, k_pos, v_val, v_pos):
    bf16 = ml_dtypes.bfloat16
    # [B,L,H,2E] -> [B,H,2E,L] -> [16, 128, L]
    qcat = np.concatenate([q_val, q_pos], axis=-1).transpose(0, 2, 3, 1)
    qcatT = np.ascontiguousarray(qcat.reshape(B * H, 2 * E, L)).astype(bf16)
    kcat = np.concatenate([k_val, k_pos], axis=-1).transpose(0, 2, 3, 1)
    kcatT = np.ascontiguousarray(kcat.reshape(B * H, 2 * E, S)).astype(bf16)
    # [B,S,H,2E] -> [B,H,S,2E] -> [16, NT, 128, 128] -> [16, 128(s), NT, 128(e)]
    vcat = np.concatenate([v_val, v_pos], axis=-1).transpose(0, 2, 1, 3)
    vtiles = vcat.reshape(B * H, NT, 128, 2 * E).transpose(0, 2, 1, 3)
    vtiles = np.ascontiguousarray(vtiles).astype(bf16)
    return [
        {
            "qT": np.ascontiguousarray(qcatT[2 * c: 2 * c + 2]),
            "kT": np.ascontiguousarray(kcatT[2 * c: 2 * c + 2]),
            "vc": np.ascontiguousarray(vtiles[2 * c: 2 * c + 2]),
        }
        for c in range(NCORES)
    ]


def kernel(q_val, q_pos, k_val, k_pos, v_val, v_pos, attn_mask=None):
    q_val, q_pos, k_val, k_pos, v_val, v_pos = (
        np.asarray(x, dtype=np.float32)
        for x in (q_val, q_pos, k_val, k_pos, v_val, v_pos)
    )
    nc = _get_nc()
    in_maps = _prep_inputs(q_val, q_pos, k_val, k_pos, v_val, v_pos)
    res = run_bass_kernel_spmd(nc, in_maps, core_ids=list(range(NCORES)))
    _CACHE["last_results"] = res
    outs = np.stack([np.asarray(r["out"], dtype=np.float32) for r in res.results])
    # [8, NPAIR, 128, L] -> [16, 128, L] -> [B, H, L, 128]
    o = outs.reshape(B * H, 2 * E, L).transpose(0, 2, 1).reshape(B, H, L, 2 * E)
    V = np.ascontiguousarray(o[..., :E].transpose(0, 2, 1, 3), dtype=np.float32)
    Vp = np.ascontiguousarray(o[..., E:].transpose(0, 2, 1, 3), dtype=np.float32)
    return (V, Vp, None)


# revision 24
# speedup vs baseline: 1.1269x; 1.0074x over previous
"""Dual-score attention kernel for Trainium2 (8 NeuronCores).

Computes, for inputs q_val/q_pos [B,L,H,E], k_val/k_pos/v_val/v_pos [B,S,H,E]:
    scores = einsum('blhe,bshe->bhls', q_val, k_val)
           + einsum('blhe,bshe->bhls', q_pos, k_pos)
    A  = softmax(scores / sqrt(E), axis=-1)
    V  = einsum('bhls,bshe->blhe', A, v_val)
    Vp = einsum('bhls,bshe->blhe', A, v_pos)
    returns (V, Vp, None)          # attn_mask is (faithfully) ignored

Sharding: the 16 (b,h) pairs are independent; each of the 8 cores handles 2.

Device-side layout trick: val/pos are concatenated along E (64+64=128) so the
dual-score sum is a single K=128 matmul. Scores are computed transposed
(St[s,l]) so that the softmax reduction over s lands on the PE partition dim,
where it is computed by a ones-vector matmul, and so that P=exp(St) is already
in the right layout to be the moving operand of the A@V matmul (stationary
Vcat = [v_val | v_pos], output [e'=128, l] accumulated over s tiles in PSUM).
"""

import os
import sys

import numpy as np

for _p in ("/opt/trn_rl_repo", "/root/.axon_site/_ro/trn_rl_repo"):
    if os.path.isdir(_p) and _p not in sys.path:
        sys.path.append(_p)

import ml_dtypes

import concourse.bass as bass
import concourse.tile as tile
from concourse import bacc, mybir
from concourse.bass_utils import run_bass_kernel_spmd

B, L, S, H, E = 2, 2048, 2048, 8, 64
NCORES = 8
NPAIR = 2          # (b,h) pairs per core
NT = S // 128      # 16 s-tiles
LH = 2             # l halves (PSUM budget)
LHW = L // LH      # 1024
NB = 512           # matmul free-dim chunk (one PSUM bank fp32)
SCALE = 1.0 / float(np.sqrt(E))

BF16 = mybir.dt.bfloat16
F32 = mybir.dt.float32

_CACHE = {}


def _build():
    nc = bacc.Bacc("TRN2", target_bir_lowering=False)

    qT = nc.dram_tensor("qT", [NPAIR, 128, L], BF16, kind="ExternalInput")
    kT = nc.dram_tensor("kT", [NPAIR, 128, S], BF16, kind="ExternalInput")
    vc = nc.dram_tensor("vc", [NPAIR, 128, NT, 128], BF16, kind="ExternalInput")
    out = nc.dram_tensor("out", [NPAIR, 128, L], F32, kind="ExternalOutput")

    with tile.TileContext(nc) as tc:
        with (
            tc.tile_pool(name="consts", bufs=1) as consts,
            tc.tile_pool(name="qk", bufs=2) as qk,
            tc.tile_pool(name="vpool", bufs=2) as vpool,
            tc.tile_pool(name="ppool", bufs=18) as ppool,
            tc.tile_pool(name="tailpool", bufs=2) as tailpool,
            tc.tile_pool(name="opool", bufs=2) as opool,
            tc.tile_pool(name="st_ps", bufs=2, space="PSUM") as st_ps,
            tc.tile_pool(name="pv_ps", bufs=2, space="PSUM") as pv_ps,
            tc.tile_pool(name="rs_ps", bufs=2, space="PSUM") as rs_ps,
        ):
            ones = consts.tile([128, 1], BF16, tag="ones")
            nc.vector.memset(ones[:], 1.0)
            # HAM warmup: junk matmuls keep the PE busy during the initial
            # DMA wait so real matmuls start at 2.4 GHz instead of 1.2.
            wz = consts.tile([128, 128], BF16, tag="wz")
            nc.vector.memset(wz[:], 0.0)
            for w in range(12):
                warm = rs_ps.tile([1, 128], F32, tag="rs", name=f"warm{w}")
                nc.tensor.matmul(warm[:], ones[:], wz[:], start=True, stop=True)

            # input tiles for both pairs, loaded upfront (double-buffered pools)
            qts, kts, vts = {}, {}, {}
            for pr in range(NPAIR):
                qt = qk.tile([128, L], BF16, tag="qt", name=f"qt{pr}")
                kt = qk.tile([128, S], BF16, tag="kt", name=f"kt{pr}")
                vt = vpool.tile([128, NT, 128], BF16, tag="vt", name=f"vt{pr}")
                qts[pr], kts[pr], vts[pr] = qt, kt, vt
                # one queue, ordered by first use: kt c0 + qt c0 unblock the
                # first scores matmul ~2us in; vt halves arrive before the
                # pipelined pv pass reaches them
                nc.sync.dma_start(out=kt[:, 0:512], in_=kT[pr, :, 0:512])
                nc.sync.dma_start(out=qt[:, 0:LHW], in_=qT[pr, :, 0:LHW])
                nc.sync.dma_start(out=vt[:, 0:8, :], in_=vc[pr, :, 0:8, :])
                nc.sync.dma_start(out=kt[:, 512:1024], in_=kT[pr, :, 512:1024])
                nc.sync.dma_start(out=vt[:, 8:16, :], in_=vc[pr, :, 8:16, :])
                for c in range(2, 4):
                    nc.sync.dma_start(
                        out=kt[:, c * 512:(c + 1) * 512],
                        in_=kT[pr, :, c * 512:(c + 1) * 512],
                    )
                nc.sync.dma_start(out=qt[:, LHW:L], in_=qT[pr, :, LHW:L])

            # Global flat software pipeline over 64 (unit, i) slots.
            # Per slot: rowsum(g-4), pv(g-3), scores(g) — scores LAST so the
            # one genuinely coupled wait (st slot freed by exp(i-2)) lands
            # after the slot's independent work, and the ACT chain paces the
            # kernel (~1.28us/slot) with the PE saturated underneath.
            units = [(pr, lh) for pr in range(NPAIR) for lh in range(LH)]
            G = len(units) * NT
            PV_LAG, RS_LAG = 4, 3

            pvt, rst, pts = {}, {}, {}
            rrept, pvct = {}, {}

            def emit_sc(g):
                u, i = divmod(g, NT)
                pr, lh = units[u]
                p_i = ppool.tile([128, LHW], BF16, tag="p", name=f"p{u}_{i}")
                pts[(u, i)] = p_i
                st = st_ps.tile([128, LHW], F32, tag="st", name=f"st{u}_{i}")
                for c in range(2):
                    nc.tensor.matmul(
                        st[:, c * NB:(c + 1) * NB],
                        kts[pr][:, i * 128:(i + 1) * 128],
                        qts[pr][:, lh * LHW + c * NB: lh * LHW + (c + 1) * NB],
                        start=True,
                        stop=True,
                    )
                nc.scalar.activation(
                    p_i[:], st[:], mybir.ActivationFunctionType.Exp, scale=SCALE,
                )

            def emit_pv(g):
                u, i = divmod(g, NT)
                pr, _ = units[u]
                if i == 0:
                    pvt[u] = [
                        pv_ps.tile([128, NB], F32, tag="pv", name=f"pv{u}{c}")
                        for c in range(2)
                    ]
                for c in range(2):
                    nc.tensor.matmul(
                        pvt[u][c][:],
                        vts[pr][:, i, :],
                        pts[(u, i)][:, c * NB:(c + 1) * NB],
                        start=(i == 0),
                        stop=(i == NT - 1),
                    )


            def emit_rs(g):
                u, i = divmod(g, NT)
                if i == 0:
                    rst[u] = [
                        rs_ps.tile([1, NB], F32, tag="rs", name=f"rs{u}{c}")
                        for c in range(2)
                    ]
                for c in range(2):
                    nc.tensor.matmul(
                        rst[u][c][:],
                        ones[:],
                        pts[(u, i)][:, c * NB:(c + 1) * NB],
                        start=(i == 0),
                        stop=(i == NT - 1),
                    )
                if i == NT - 1:
                    recip = tailpool.tile([1, LHW], F32, tag="recip",
                                          name=f"recip{u}")
                    rrep = tailpool.tile([128, LHW], F32, tag="rrep",
                                         name=f"rrep{u}")
                    for c in range(2):
                        nc.vector.reciprocal_approx_fast(
                            out=recip[:, c * NB:(c + 1) * NB], in_=rst[u][c][:]
                        )
                        nc.gpsimd.partition_broadcast(
                            rrep[:, c * NB:(c + 1) * NB],
                            recip[:, c * NB:(c + 1) * NB],
                        )
                    rrept[u] = rrep

            def emit_tail(u):
                # per-chunk copy->mult->dma chain; the copy frees the pv
                # PSUM banks for the next unit. The last unit has no
                # successor, so it multiplies straight from PSUM.
                pr, lh = units[u]
                last = u == len(units) - 1
                o_sb = opool.tile([128, LHW], F32, tag="o", name=f"o{u}")
                if not last:
                    pvc = tailpool.tile([128, LHW], F32, tag="pvc",
                                        name=f"pvc{u}")
                for c in range(2):
                    sl = slice(c * NB, (c + 1) * NB)
                    if last:
                        nc.vector.tensor_mul(o_sb[:, sl], pvt[u][c][:],
                                             rrept[u][:, sl])
                    else:
                        nc.vector.tensor_copy(pvc[:, sl], pvt[u][c][:])
                        nc.vector.tensor_mul(o_sb[:, sl], pvc[:, sl],
                                             rrept[u][:, sl])
                    nc.sync.dma_start(
                        out=out[pr, :, lh * LHW + c * NB: lh * LHW + (c + 1) * NB],
                        in_=o_sb[:, sl],
                    )

            for g in range(G + PV_LAG + 1):
                h = g - RS_LAG
                # pair the rowsums of two adjacent slots: 4 consecutive
                # matmuls share the `ones` stationary -> one fewer weight
                # swap per 2 slots
                if 0 <= h < G and h % 2 == 0:
                    emit_rs(h)
                    emit_rs(h + 1)
                h = g - PV_LAG
                if 0 <= h < G:
                    emit_pv(h)
                    u, i = divmod(h, NT)
                    if i == NT - 1:
                        emit_tail(u)
                # scores also paired: one st-slot wait per two slots
                if g < G and g % 2 == 0:
                    emit_sc(g)
                    emit_sc(g + 1)

    nc.compile()
    return nc


def _get_nc():
    if "nc" not in _CACHE:
        _CACHE["nc"] = _build()
    return _CACHE["nc"]


def _prep_inputs(q_val, q_pos, k_val, k_pos, v_val, v_pos):
    bf16 = ml_dtypes.bfloat16
    # [B,L,H,2E] -> [B,H,2E,L] -> [16, 128, L]
    qcat = np.concatenate([q_val, q_pos], axis=-1).transpose(0, 2, 3, 1)
    qcatT = np.ascontiguousarray(qcat.reshape(B * H, 2 * E, L)).astype(bf16)
    kcat = np.concatenate([k_val, k_pos], axis=-1).transpose(0, 2, 3, 1)
    kcatT = np.ascontiguousarray(kcat.reshape(B * H, 2 * E, S)).astype(bf16)
    # [B,S,H,2E] -> [B,H,S,2E] -> [16, NT, 128, 128] -> [16, 128(s), NT, 128(e)]
    vcat = np.concatenate([v_val, v_pos], axis=-1).transpose(0, 2, 1, 3)
    vtiles = vcat.reshape(B * H, NT, 128, 2 * E).transpose(0, 2, 1, 3)
    vtiles = np.ascontiguousarray(vtiles).astype(bf16)
    return [
        {
            "qT": np.ascontiguousarray(qcatT[2 * c: 2 * c + 2]),
            "kT": np.ascontiguousarray(kcatT[2 * c: 2 * c + 2]),
            "vc": np.ascontiguousarray(vtiles[2 * c: 2 * c + 2]),
        }
        for c in range(NCORES)
    ]


def kernel(q_val, q_pos, k_val, k_pos, v_val, v_pos, attn_mask=None):
    q_val, q_pos, k_val, k_pos, v_val, v_pos = (
        np.asarray(x, dtype=np.float32)
        for x in (q_val, q_pos, k_val, k_pos, v_val, v_pos)
    )
    nc = _get_nc()
    in_maps = _prep_inputs(q_val, q_pos, k_val, k_pos, v_val, v_pos)
    res = run_bass_kernel_spmd(nc, in_maps, core_ids=list(range(NCORES)))
    _CACHE["last_results"] = res
    outs = np.stack([np.asarray(r["out"], dtype=np.float32) for r in res.results])
    # [8, NPAIR, 128, L] -> [16, 128, L] -> [B, H, L, 128]
    o = outs.reshape(B * H, 2 * E, L).transpose(0, 2, 1).reshape(B, H, L, 2 * E)
    V = np.ascontiguousarray(o[..., :E].transpose(0, 2, 1, 3), dtype=np.float32)
    Vp = np.ascontiguousarray(o[..., E:].transpose(0, 2, 1, 3), dtype=np.float32)
    return (V, Vp, None)
